# revision 22
# baseline (speedup 1.0000x reference)
"""AttentionLSTM Trainium2 kernel (v2).

Strategy: data-parallel over batch across 8 NeuronCores (B=64 -> 8/core),
all weights replicated. Mixed precision: bf16 matmul operands, fp32 PSUM
accumulation and fp32 cell state.

Changes vs the original baseline:
  - Embedding gather done on HOST (ship embT [128,4,2048] bf16 = 2MB/core
    instead of the replicated 33MB table); device gather phase removed.
  - Recurrence loop unrolled 8 timesteps per For_i iteration (back-edge
    barrier cost amortized 8x).
  - Gate-major weight order [g,i,f,o]; each gate gets its own PSUM bank so
    gate nonlinearities overlap the matmul stream of later gates. All
    in-loop accesses are static (ping-pong h buffers, per-iteration staged
    XS/HSTG transfers) -- dynamic APs in unrolled hw loops exhaust engine
    registers.
  - Real Sigmoid ACT table (preloaded with Tanh before the loop) instead of
    the tanh(x/2) affine trick; cell-state chain on GpSimd so the Tile
    scheduler cannot defer it behind the o-gate matmuls.

Per-core pipeline:
  P1  L0 input projection (bulk matmul from embT) -> x-proj chunks in DRAM
  P2  L0 recurrence (weights-stationary, gates transposed layout)
  P3  L1 input projection from SBUF h_seq -> x-proj chunks in DRAM
  P4  L1 recurrence
  P5  additive attention + output head, logits [2, 8]

"""

import numpy as np

B, T, V, E, H, A, C = 64, 256, 32000, 512, 1024, 512, 2
NCORES, BL = 8, 8
MT = 32          # m-tiles over 4H gate rows
KT = 8           # k-tiles over H contraction
KT0 = 4          # k-tiles over E contraction (layer-0 inproj)
NCHUNK = 2       # time chunks
CSTEP = 128      # steps per chunk
UNROLL = 8       # timesteps per For_i iteration
GPERM = [2, 0, 1, 3]   # gate row reorder: [g, i, f, o]

_CACHE = {}


def _bf16(a):
    import ml_dtypes
    return np.ascontiguousarray(np.asarray(a, np.float32)).astype(ml_dtypes.bfloat16)


def _f32(a):
    return np.ascontiguousarray(a, np.float32)


def _f8(a):
    import ml_dtypes
    return np.ascontiguousarray(np.asarray(a, np.float32)).astype(
        ml_dtypes.float8_e4m3
    )


def _tiles(w, mt, kt):
    """w: [mt*128, kt*128] row-major weight (already gate-reordered).
    Returns [128, mt*kt, 128] where block j = m*kt + k holds w[mK:mK+128,
    kK:kK+128].T  (lhsT tile: partitions = contraction dim)."""
    m4 = w.reshape(mt, 128, kt, 128)          # [m, q, k, p]
    return np.ascontiguousarray(np.transpose(m4, (3, 0, 2, 1))).reshape(
        128, mt * kt, 128
    )


def _gate_reorder(w):
    """Reorder rows of a [4H, D] matrix from [i,f,g,o] to [g,i,f,o]."""
    g = w.reshape(4, H, -1)
    return np.concatenate([g[p] for p in GPERM], 0)


def _prep_shared(inputs):
    """Host-side weight prep (identical for all cores)."""
    sh = {}
    for L, (wi, wh, bi, bh, kt) in {
        0: ("w_ih0", "w_hh0", "b_ih0", "b_hh0", KT0),
        1: ("w_ih1", "w_hh1", "b_ih1", "b_hh1", KT),
    }.items():
        wir = _gate_reorder(_f32(inputs[wi]))
        whr = _gate_reorder(_f32(inputs[wh]))
        br = _gate_reorder((_f32(inputs[bi]) + _f32(inputs[bh]))[:, None])[:, 0]
        sh[f"wi{L}"] = _bf16(_tiles(wir, MT, kt))
        sh[f"wh{L}"] = _bf16(_tiles(whr, MT, KT))
        sh[f"b{L}"] = _f32(br.reshape(MT, 128).T)           # [128, 32]

    sh["m1w"] = _bf16(_tiles(_f32(inputs["m1_w"]), 4, KT))  # [128, 32, 128]
    sh["m2w"] = _bf16(_tiles(_f32(inputs["m2_w"]), 4, KT))
    sh["m1b"] = _f32(_f32(inputs["m1_b"]).reshape(4, 128).T)   # [128, 4]
    sh["m2b"] = _f32(_f32(inputs["m2_b"]).reshape(4, 128).T)
    sh["vT"] = _bf16(_f32(inputs["v"]).reshape(4, 128).T)      # [128, 4]
    sh["nw"] = _bf16(_tiles(_f32(inputs["n_w"]), 8, 16))       # [128, 128, 128]
    sh["nb"] = _f32(_f32(inputs["n_b"]).reshape(8, 128).T)     # [128, 8]
    ow = _f32(inputs["out_w"]).T                                # [H, 2]
    sh["ow"] = _bf16(ow.reshape(8, 128, 2).transpose(1, 0, 2))  # [128, 8, 2]
    sh["ob"] = _f32(_f32(inputs["out_b"]).reshape(2, 1))        # [2, 1]
    return sh


def _install_drain_patch():
    from concourse.tile import TileContext, ScopedClock

    if getattr(TileContext, "_drain_patched", False):
        return

    def _patched(self, tick_clock, wait_clock):
        drain_inst = self.nc.sync.drain()
        wait_clock.add_sem_waits(
            drain_inst.ins, ScopedClock({None: tick_clock.global_clock})
        )
        si = drain_inst.ins.sync_info
        waits = list(si.on_wait)
        if len(waits) > 1:
            si.on_wait = waits[:1]
            for w in waits[1:]:
                d2 = self.nc.sync.drain()
                wait_clock.add_sem_waits(
                    d2.ins, ScopedClock({None: tick_clock.global_clock})
                )
                d2.ins.sync_info.on_wait = [w]
        self.nc.all_engine_barrier()
        popped = self.nc._tile_sem_poison_stack.pop()
        assert popped is self._sem_poison
        self.nc.clear_and_free_semaphores(list(self.sems.allocated().values()))
        self.nc.all_engine_barrier()

    TileContext._drain_and_barrier = _patched
    TileContext._drain_patched = True


def _split_waits(nc, limit=1):
    """This container's walrus build rejects instructions with more than
    ~2 sync waits. Move excess waits onto same-engine NoOps inserted
    immediately before the instruction."""
    import copy
    import concourse.mybir as mybir

    n_split = 0
    for f in nc.m.functions:
        for bb in f.blocks:
            li = list(bb.instructions)
            out = []
            for inst in li:
                si = inst.sync_info
                if si is not None and len(si.on_wait) > limit:
                    waits = list(si.on_wait)
                    for j in range(0, len(waits) - limit, limit):
                        nop = mybir.InstNoOp(
                            name=f"{inst.name}_ws{j}", ins=[], outs=[]
                        )
                        nop.engine = inst.engine
                        si2 = copy.copy(si)
                        si2.on_wait = waits[j : j + limit]
                        si2.on_update = []
                        nop.sync_info = si2
                        out.append(nop)
                        n_split += 1
                    si.on_wait = waits[len(waits) - limit :]
                inst.sync_info = si
                out.append(inst)
            bb.instructions = out
    return n_split


def _build_program():
    import concourse.bass as bass
    import concourse.mybir as mybir
    from concourse.bass import ds
    from concourse.tile import TileContext
    from concourse.masks import make_identity

    _install_drain_patch()

    dt = mybir.dt
    AF = mybir.ActivationFunctionType
    OP = mybir.AluOpType
    AX = mybir.AxisListType
    PE = mybir.EngineType.PE
    DVE = mybir.EngineType.DVE
    ACT = mybir.EngineType.Activation

    nc = bass.Bass()

    # ---- DRAM I/O ----
    embT_d = nc.dram_tensor("embT", [128, KT0, T * BL], dt.bfloat16, kind="ExternalInput")
    wi0_d = nc.dram_tensor("wi0", [128, MT * KT0, 128], dt.bfloat16, kind="ExternalInput")
    wi1_d = nc.dram_tensor("wi1", [128, MT * KT, 128], dt.bfloat16, kind="ExternalInput")
    wh0_d = nc.dram_tensor("wh0", [128, MT * KT, 128], dt.bfloat16, kind="ExternalInput")
    wh1_d = nc.dram_tensor("wh1", [128, MT * KT, 128], dt.bfloat16, kind="ExternalInput")
    b0_d = nc.dram_tensor("b0", [128, MT], dt.float32, kind="ExternalInput")
    b1_d = nc.dram_tensor("b1", [128, MT], dt.float32, kind="ExternalInput")
    m1w_d = nc.dram_tensor("m1w", [128, 4 * KT, 128], dt.bfloat16, kind="ExternalInput")
    m2w_d = nc.dram_tensor("m2w", [128, 4 * KT, 128], dt.bfloat16, kind="ExternalInput")
    m1b_d = nc.dram_tensor("m1b", [128, 4], dt.float32, kind="ExternalInput")
    m2b_d = nc.dram_tensor("m2b", [128, 4], dt.float32, kind="ExternalInput")
    vT_d = nc.dram_tensor("vT", [128, 4], dt.bfloat16, kind="ExternalInput")
    nw_d = nc.dram_tensor("nw", [128, 8 * 16, 128], dt.bfloat16, kind="ExternalInput")
    nb_d = nc.dram_tensor("nb", [128, 8], dt.float32, kind="ExternalInput")
    ow_d = nc.dram_tensor("ow", [128, 8, 2], dt.bfloat16, kind="ExternalInput")
    ob_d = nc.dram_tensor("ob", [2, 1], dt.float32, kind="ExternalInput")
    out_d = nc.dram_tensor("out_t", [2, BL], dt.float32, kind="ExternalOutput")

    SEQ = (T + 1) * BL  # hidden-seq columns: block 0 = zero initial state

    with TileContext(nc) as tc:
        with (
            tc.tile_pool(name="dram", bufs=1, space="DRAM") as dpool,
            tc.tile_pool(name="glob", bufs=1) as gp,
        ):
            xp_dram = dpool.tile([NCHUNK, 128, MT, CSTEP * BL], dt.bfloat16)
            # shared L0-out / L1-out hidden sequence  [128, k, (t+1)*8+b] bf16
            h_seq = gp.tile([128, KT, SEQ], dt.bfloat16)
            nc.vector.memset(h_seq[:, :, 0:BL], 0.0)
            b0_s = gp.tile([128, MT], dt.float32)
            nc.sync.dma_start(b0_s[:], b0_d[:])
            b1_s = gp.tile([128, MT], dt.float32)
            nc.sync.dma_start(b1_s[:], b1_d[:])
            embT = gp.tile([128, KT0, T * BL], dt.bfloat16)
            nc.sync.dma_start(embT[:], embT_d[:])
            # keep the Tanh ACT table resident before every recurrence loop
            dummy = gp.tile([128, 1], dt.float32)
            nc.vector.memset(dummy[:], 0.0)

            # ---------- P1: L0 input projection ----------
            with (
                tc.tile_pool(name="wi0p", bufs=1) as p1,
                tc.tile_pool(name="xps", bufs=2) as xps,
                tc.tile_pool(name="pp1", bufs=2, space="PSUM") as pp1,
            ):
                wi0 = p1.tile([128, MT * KT0, 128], dt.bfloat16)
                nc.sync.dma_start(wi0[:], wi0_d[:])
                for m in range(MT):
                    xp = xps.tile([128, T * BL], dt.bfloat16, tag="xp")
                    for nn in range(4):
                        ps = pp1.tile([128, 512], dt.float32, tag="mm")
                        for k in range(KT0):
                            nc.tensor.matmul(
                                ps[:],
                                wi0[:, m * KT0 + k, :],
                                embT[:, k, nn * 512 : (nn + 1) * 512],
                                start=(k == 0),
                                stop=(k == KT0 - 1),
                            )
                        nc.scalar.activation(
                            xp[:, nn * 512 : (nn + 1) * 512],
                            ps[:],
                            AF.Identity,
                            bias=b0_s[:, m : m + 1],
                        )
                    for cpart in range(NCHUNK):
                        nc.sync.dma_start(
                            xp_dram[cpart, :, m, :],
                            xp[:, cpart * 1024 : (cpart + 1) * 1024],
                        )

            # ---------- recurrence helper ----------
            def recurrence(wh_dram, layer):
                with (
                    tc.tile_pool(name=f"wh{layer}", bufs=1) as wp,
                    tc.tile_pool(name=f"rec{layer}", bufs=1) as rp,
                    tc.tile_pool(name=f"xpc{layer}", bufs=1) as xpp,
                    tc.tile_pool(name=f"rp{layer}", bufs=1, space="PSUM") as rpp,
                ):
                    w = wp.tile([128, MT * KT, 128], dt.bfloat16, tag="w")
                    nc.sync.dma_start(w[:], wh_dram[:])
                    cst = rp.tile([128, 64], dt.float32, tag="c")
                    nc.vector.memset(cst[:], 0.0)
                    ident = rp.tile([128, 128], dt.bfloat16, tag="id")
                    make_identity(nc, ident[:])
                    # load the Tanh + Sigmoid tables before entering the loop
                    # so in-loop activations don't trigger per-iteration loads
                    nc.scalar.activation(dummy[:], dummy[:], AF.Tanh)
                    nc.scalar.activation(dummy[:], dummy[:], AF.Sigmoid)

                    # static ping-pong h buffers (parity = timestep % 2) and
                    # per-iteration staging tiles; all in-loop accesses are
                    # static except one XS stage-in and one HSTG write-back
                    # per iteration (register-pressure limit on dynamic APs)
                    H = [rp.tile([128, 64], dt.bfloat16, tag=f"H{s}",
                                 name=f"H{s}") for s in range(2)]
                    nc.vector.memset(H[0][:], 0.0)
                    XS = rp.tile([128, MT, BL * UNROLL], dt.bfloat16, tag="XS")
                    HSTG = rp.tile([128, KT, BL * UNROLL], dt.bfloat16, tag="HS")
                    # g/i/f psum: one bank each (their DVE reads happen
                    # after the PE has moved on to the next gate's bank, so
                    # no intra-bank read-while-write). o-gate: two banks,
                    # chunks alternate between them so the incremental chunk
                    # reads never share a bank with in-flight PE writes.
                    PSG = rpp.tile([128, 512], dt.float32, tag="psg", name="psg")
                    PSI = rpp.tile([128, 512], dt.float32, tag="psi", name="psi")
                    PSF = rpp.tile([128, 512], dt.float32, tag="psf", name="psf")
                    POA = rpp.tile([128, 512], dt.float32, tag="poa", name="poa")
                    POB = rpp.tile([128, 512], dt.float32, tag="pob", name="pob")
                    PO = [POA, POB]
                    GB, TG, SG, T1, C2, TCS = [], [], [], [], [], []
                    for s in range(2):
                        GB.append(rp.tile([128, 256], dt.float32,
                                          tag=f"gb{s}", name=f"gb{s}"))
                        TG.append(rp.tile([128, 64], dt.float32,
                                          tag=f"tg{s}", name=f"tg{s}"))
                        SG.append(rp.tile([128, 192], dt.float32,
                                          tag=f"sg{s}", name=f"sg{s}"))
                        T1.append(rp.tile([128, 64], dt.float32,
                                          tag=f"t1{s}", name=f"t1{s}"))
                        C2.append(rp.tile([128, 64], dt.float32,
                                          tag=f"c2{s}", name=f"c2{s}"))
                        TCS.append(rp.tile([128, 64], dt.float32,
                                           tag=f"tc{s}", name=f"tc{s}"))

                    def step(u):
                        """One timestep, fully static; u = position within
                        the unrolled iteration, parity s = u % 2."""
                        s = u % 2
                        pss = [PSG, PSI, PSF, None]
                        gbuf, tg, sig = GB[s], TG[s], SG[s]
                        t1, c2, tcs = T1[s], C2[s], TCS[s]
                        us = slice(u * BL, (u + 1) * BL)

                        for g in range(4):
                            ps = pss[g]
                            for m in range(8):
                                mt = g * 8 + m
                                if g == 3:
                                    # chunk c = m // 2; bank alternates per
                                    # chunk; 16 cols per chunk within bank
                                    c = m // 2
                                    ps = PO[c % 2]
                                    pcol = (c // 2) * 16 + (m % 2) * BL
                                else:
                                    pcol = m * BL
                                xp_in_psum = g == 3 and m >= 6
                                for k in range(KT):
                                    nc.tensor.matmul(
                                        ps[:, pcol : pcol + BL],
                                        w[:, mt * KT + k, :],
                                        H[s][:, k * BL : (k + 1) * BL],
                                        start=(k == 0),
                                        stop=(k == KT - 1 and not xp_in_psum),
                                    )
                                if xp_in_psum:
                                    # fold xp into PSUM so the last chunk's
                                    # sigmoid reads PSUM directly -- removes
                                    # the DVE add from the end-of-step
                                    # critical chain
                                    nc.tensor.matmul(
                                        ps[:, pcol : pcol + BL],
                                        ident[:],
                                        XS[:, 24 + m, us],
                                        start=False,
                                        stop=True,
                                    )
                                if g == 3 and m % 2 == 1:
                                    # o-gate processed in 16-col chunks as
                                    # soon as each pair of m-tiles lands, so
                                    # h is produced incrementally and the
                                    # next step's matmuls start with almost
                                    # no tail wait
                                    c0 = (m - 1) * BL
                                    cs = slice(c0, c0 + 2 * BL)
                                    pc0 = (c // 2) * 16
                                    so = sig[:, 128 + c0 : 128 + c0 + 16]
                                    if m == 7:
                                        nc.scalar.activation(
                                            so, ps[:, pc0 : pc0 + 16],
                                            AF.Sigmoid,
                                        )
                                    else:
                                        go = gbuf[:, 192 + c0 : 192 + c0 + 16]
                                        nc.vector.tensor_tensor(
                                            out=go.rearrange("p (m b) -> p m b", b=BL),
                                            in0=ps[:, pc0 : pc0 + 16].rearrange(
                                                "p (m b) -> p m b", b=BL),
                                            in1=XS[:, 24 + m - 1 : 24 + m + 1, us],
                                            op=OP.add,
                                        )
                                        nc.scalar.activation(so, go, AF.Sigmoid)
                                    nc.vector.tensor_tensor(
                                        out=H[1 - s][:, cs],
                                        in0=so,
                                        in1=tcs[:, cs],
                                        op=OP.mult,
                                    )
                            if g == 3:
                                continue
                            # gate pre-activation = psum + xp  (overlaps
                            # with the next gate's matmul stream)
                            gv = gbuf[:, g * 64 : (g + 1) * 64]
                            nc.vector.tensor_tensor(
                                out=gv.rearrange("p (m b) -> p m b", b=BL),
                                in0=ps[:, 0:64].rearrange("p (m b) -> p m b", b=BL),
                                in1=XS[:, g * 8 : (g + 1) * 8, us],
                                op=OP.add,
                            )
                            if g == 0:  # g-gate: tanh
                                nc.scalar.activation(tg[:], gv, AF.Tanh)
                            else:       # i, f: sigmoid (own ACT table)
                                sl = sig[:, (g - 1) * 64 : g * 64]
                                nc.scalar.activation(sl, gv, AF.Sigmoid)
                            if g == 1:
                                # cell-state chain on GpSimd: it is idle and
                                # has no queue-order conflicts, so these run
                                # as soon as deps resolve (overlapped with
                                # the o-gate matmul stream)
                                nc.gpsimd.tensor_tensor(
                                    out=t1[:], in0=sig[:, 0:64], in1=tg[:],
                                    op=OP.mult,
                                )
                            if g == 2:
                                nc.gpsimd.tensor_tensor(
                                    out=c2[:], in0=sig[:, 64:128], in1=cst[:],
                                    op=OP.mult,
                                )
                                nc.gpsimd.tensor_tensor(
                                    out=cst[:], in0=c2[:], in1=t1[:], op=OP.add
                                )
                                nc.scalar.activation(tcs[:], cst[:], AF.Tanh)
                        nc.vector.tensor_copy(
                            HSTG[:, :, us],
                            H[1 - s][:].rearrange("p (k b) -> p k b", b=BL),
                        )

                    for cpart in range(NCHUNK):
                        xpc = xpp.tile([128, MT, CSTEP * BL], dt.bfloat16, tag="xpc")
                        nc.sync.dma_start(xpc[:], xp_dram[cpart])
                        base = cpart * CSTEP * BL
                        seq_w = h_seq[:, :, base + BL :]
                        with tc.For_i(
                            0, CSTEP * BL, BL * UNROLL, hint_engines=(PE,)
                        ) as iv:
                            nc.vector.tensor_copy(
                                XS[:].rearrange("p m (u b) -> p m (u b)", b=BL),
                                xpc[:, :, ds(iv, BL * UNROLL)],
                            )
                            for u in range(UNROLL):
                                step(u)
                            nc.vector.tensor_copy(
                                seq_w[:, :, ds(iv, BL * UNROLL)], HSTG[:]
                            )

            # ---------- P2: L0 recurrence ----------
            recurrence(wh0_d, 0)

            # ---------- P3: L1 input projection (from SBUF h_seq) ----------
            with (
                tc.tile_pool(name="wi1p", bufs=1) as p3,
                tc.tile_pool(name="xps1", bufs=2) as xps1,
                tc.tile_pool(name="pp3", bufs=2, space="PSUM") as pp3,
            ):
                wi1 = p3.tile([128, MT * KT, 128], dt.bfloat16)
                nc.sync.dma_start(wi1[:], wi1_d[:])
                for m in range(MT):
                    xp = xps1.tile([128, T * BL], dt.bfloat16, tag="xp1")
                    for nn in range(4):
                        ps = pp3.tile([128, 512], dt.float32, tag="mm1")
                        for k in range(KT):
                            nc.tensor.matmul(
                                ps[:],
                                wi1[:, m * KT + k, :],
                                h_seq[:, k, BL + nn * 512 : BL + (nn + 1) * 512],
                                start=(k == 0),
                                stop=(k == KT - 1),
                            )
                        nc.scalar.activation(
                            xp[:, nn * 512 : (nn + 1) * 512],
                            ps[:],
                            AF.Identity,
                            bias=b1_s[:, m : m + 1],
                        )
                    for cpart in range(NCHUNK):
                        nc.sync.dma_start(
                            xp_dram[cpart, :, m, :],
                            xp[:, cpart * 1024 : (cpart + 1) * 1024],
                        )

            # ---------- P4: L1 recurrence ----------
            recurrence(wh1_d, 1)

            # ---------- P5: attention + head ----------
            with (
                tc.tile_pool(name="att", bufs=1) as at,
                tc.tile_pool(name="attm", bufs=2) as atm,
                tc.tile_pool(name="ap1", bufs=2, space="PSUM") as ap1,
                tc.tile_pool(name="ap2", bufs=2, space="PSUM") as ap2,
                tc.tile_pool(name="ap3", bufs=1, space="PSUM") as ap3,
            ):
                m1w = at.tile([128, 4 * KT, 128], dt.bfloat16)
                nc.sync.dma_start(m1w[:], m1w_d[:])
                m2w = at.tile([128, 4 * KT, 128], dt.bfloat16)
                nc.sync.dma_start(m2w[:], m2w_d[:])
                m1b = at.tile([128, 4], dt.float32)
                nc.sync.dma_start(m1b[:], m1b_d[:])
                m2b = at.tile([128, 4], dt.float32)
                nc.sync.dma_start(m2b[:], m2b_d[:])
                vT = at.tile([128, 4], dt.bfloat16)
                nc.sync.dma_start(vT[:], vT_d[:])
                nw = at.tile([128, 8 * 16, 128], dt.bfloat16)
                nc.sync.dma_start(nw[:], nw_d[:])
                nb = at.tile([128, 8], dt.float32)
                nc.sync.dma_start(nb[:], nb_d[:])
                ow = at.tile([128, 8, 2], dt.bfloat16)
                nc.sync.dma_start(ow[:], ow_d[:])
                ob = at.tile([2, 1], dt.float32)
                nc.sync.dma_start(ob[:], ob_d[:])
                ones = at.tile([1, 128], dt.bfloat16)
                nc.vector.memset(ones[:], 1.0)

                hl = T * BL  # h_last column offset (block T)
                hv = h_seq[:, :, BL:SEQ]  # h2 sequence, blocks 1..T

                # m2T [128, 4m, 8b]
                m2T = at.tile([128, 4, BL], dt.float32)
                for m in range(4):
                    ps = ap2.tile([128, BL], dt.float32, tag="sm")
                    for k in range(KT):
                        nc.tensor.matmul(
                            ps[:],
                            m2w[:, m * KT + k, :],
                            h_seq[:, k, hl : hl + BL],
                            start=(k == 0),
                            stop=(k == KT - 1),
                        )
                    nc.scalar.activation(
                        m2T[:, m, :], ps[:], AF.Identity, bias=m2b[:, m : m + 1]
                    )

                # u = tanh(m1 + m2 + m1b), b-major cols (b*256+t), bf16
                u = at.tile([128, 4, 2048], dt.bfloat16)
                tmpu = atm.tile([128, 512], dt.float32, tag="tmpu")
                for m in range(4):
                    for nn in range(4):
                        t0 = nn * 64  # t-range within each b
                        ps = ap1.tile([128, 512], dt.float32, tag="big")
                        psv = ps[:].rearrange("p (b t) -> p b t", t=64)
                        for k in range(KT):
                            rhs = (
                                hv[:, k, :]
                                .rearrange("p (t b) -> p b t", b=8)[
                                    :, :, t0 : t0 + 64
                                ]
                            )
                            nc.tensor.matmul(
                                psv,
                                m1w[:, m * KT + k, :],
                                rhs,
                                start=(k == 0),
                                stop=(k == KT - 1),
                            )
                        tv = tmpu[:].rearrange("p (b t) -> p b t", t=64)
                        nc.vector.tensor_tensor(
                            out=tv,
                            in0=psv,
                            in1=m2T[:, m, :].to_broadcast([128, 8, 64]),
                            op=OP.add,
                        )
                        uv = u[:, m, :].rearrange("p (b t) -> p b t", t=256)[
                            :, :, t0 : t0 + 64
                        ]
                        nc.scalar.activation(
                            uv, tv, AF.Tanh, bias=m1b[:, m : m + 1]
                        )

                # scores [1, (b,t)] f32
                scores = at.tile([1, 2048], dt.float32)
                for nn in range(4):
                    ps = ap3.tile([1, 512], dt.float32, tag="sc")
                    for m in range(4):
                        nc.tensor.matmul(
                            ps[:],
                            vT[:, m : m + 1],
                            u[:, m, nn * 512 : (nn + 1) * 512],
                            start=(m == 0),
                            stop=(m == 3),
                        )
                    nc.vector.tensor_copy(scores[:, nn * 512 : (nn + 1) * 512], ps[:])

                # softmax over t (per b)
                scv = scores[:].rearrange("p (b t) -> p b t", t=256)
                mx = at.tile([1, 8], dt.float32)
                nc.vector.tensor_reduce(mx[:], scv, axis=AX.X, op=OP.max)
                nc.vector.tensor_tensor(
                    out=scv, in0=scv, in1=mx[:].to_broadcast([1, 8, 256]), op=OP.subtract
                )
                ex = at.tile([1, 2048], dt.float32)
                nc.scalar.activation(ex[:], scores[:], AF.Exp)
                exv = ex[:].rearrange("p (b t) -> p b t", t=256)
                sm = at.tile([1, 8], dt.float32)
                nc.vector.tensor_reduce(sm[:], exv, axis=AX.X, op=OP.add)
                inv = at.tile([1, 8], dt.float32)
                nc.vector.reciprocal(inv[:], sm[:])
                attn = at.tile([1, 2048], dt.bfloat16)
                nc.vector.tensor_tensor(
                    out=attn[:].rearrange("p (b t) -> p b t", t=256),
                    in0=exv,
                    in1=inv[:].to_broadcast([1, 8, 256]),
                    op=OP.mult,
                )

                # broadcast attn to 128 partitions via K=1 matmul
                attn128 = at.tile([128, 2048], dt.float32)
                for nn in range(4):
                    ps = ap1.tile([128, 512], dt.float32, tag="big")
                    nc.tensor.matmul(
                        ps[:],
                        ones[:],
                        attn[:, nn * 512 : (nn + 1) * 512],
                        start=True,
                        stop=True,
                    )
                    nc.vector.tensor_copy(attn128[:, nn * 512 : (nn + 1) * 512], ps[:])

                # context[h, b] = sum_t h2[h, (t,b)] * attn[b, t]
                ctxf = at.tile([128, KT, BL], dt.float32)
                av = attn128[:].rearrange("p (b t) -> p b t", t=256)
                for k in range(KT):
                    tmp = atm.tile([128, 2048], dt.float32, tag="ctx")
                    tv = tmp[:].rearrange("p (b t) -> p b t", t=256)
                    hvk = hv[:, k, :].rearrange("p (t b) -> p b t", b=8)
                    nc.vector.tensor_tensor(out=tv, in0=hvk, in1=av, op=OP.mult)
                    nc.vector.tensor_reduce(ctxf[:, k, :], tv, axis=AX.X, op=OP.add)
                ctx = at.tile([128, KT * BL], dt.bfloat16)
                nc.vector.tensor_copy(
                    ctx[:].rearrange("p (k b) -> p k b", b=8), ctxf[:]
                )

                # n_vec = tanh([ctx, h_last] @ n_w.T + n_b)   -> [128, 8m, 8b]
                nT = at.tile([128, 8, BL], dt.bfloat16)
                for m in range(8):
                    ps = ap2.tile([128, BL], dt.float32, tag="sm")
                    for k in range(16):
                        rhs = (
                            ctx[:, (k * BL) : (k * BL + BL)]
                            if k < 8
                            else h_seq[:, k - 8, hl : hl + BL]
                        )
                        nc.tensor.matmul(
                            ps[:],
                            nw[:, m * 16 + k, :],
                            rhs,
                            start=(k == 0),
                            stop=(k == 15),
                        )
                    nc.scalar.activation(
                        nT[:, m, :], ps[:], AF.Tanh, bias=nb[:, m : m + 1]
                    )

                # logit [2, 8]
                psl = ap3.tile([2, BL], dt.float32, tag="sc")
                for k in range(8):
                    nc.tensor.matmul(
                        psl[:],
                        ow[:, k, :],
                        nT[:, k, :],
                        start=(k == 0),
                        stop=(k == 7),
                    )
                lg = at.tile([2, BL], dt.float32)
                nc.scalar.activation(lg[:], psl[:], AF.Identity, bias=ob[:])
                nc.sync.dma_start(out_d[:], lg[:])

    _split_waits(nc)
    return nc


def kernel(**inputs):
    from concourse import bass_utils

    if "nc" not in _CACHE:
        _CACHE["nc"] = _build_program()
    nc = _CACHE["nc"]

    sh = _prep_shared(inputs)
    x = np.asarray(inputs["x"]).astype(np.int64)
    emb32 = _f32(inputs["embed_w"])

    in_maps = []
    for c in range(NCORES):
        xl = x[c * BL : (c + 1) * BL]                # [8, 256]
        xf = np.ascontiguousarray(xl.T).reshape(-1)  # t-major flat (t*8+b)
        g = emb32[xf]                                # [2048, 512]
        embT = _bf16(
            np.ascontiguousarray(g.T).reshape(KT0, 128, T * BL).transpose(1, 0, 2)
        )                                            # [128, 4, 2048]
        m = {
            "embT": embT,
            "wi0": sh["wi0"], "wi1": sh["wi1"],
            "wh0": sh["wh0"], "wh1": sh["wh1"],
            "b0": sh["b0"], "b1": sh["b1"],
            "m1w": sh["m1w"], "m2w": sh["m2w"],
            "m1b": sh["m1b"], "m2b": sh["m2b"],
            "vT": sh["vT"], "nw": sh["nw"], "nb": sh["nb"],
            "ow": sh["ow"], "ob": sh["ob"],
        }
        in_maps.append(m)

    res = bass_utils.run_bass_kernel_spmd(nc, in_maps, core_ids=list(range(NCORES)))
    out = np.zeros((B, C), np.float32)
    for c in range(NCORES):
        out[c * BL : (c + 1) * BL] = res.results[c]["out_t"].T
    return out
